# revision 24
# baseline (speedup 1.0000x reference)
"""Trainium2 Bass kernel for ConfigurableLSTMModel (RevIN + LSTM encoder +
autoregressive LSTM decoder + output projection + RevIN denorm).

Sharding: data-parallel over batch across 8 NeuronCores (B=256 -> 32/core).
Weights are replicated; the sequential scan is per-sample so batch sharding
is exact.

Per-core algorithm (B_loc=32, D=16, H=512, T_in=512, T_out=200), v2:
  - RevIN stats/normalize on-chip (layout [16(d) x 32(b) x T]).
  - Gates live in ONE [128, 512] PSUM tile: column-tiled matmuls place the
    four gates in partition quadrants [f@0, o@32, i@64, 2g@96]. bf16
    operands (fp32r forbids nonzero dst partitions) let 4 tile_position
    streams overlap on the PE -> ~1.9x faster gate GEMM than sequential.
  - All activations are SIGMOID (one ACT table, no tanh):
      tanh(x) = 2*sigmoid(2x) - 1, the 2x folded into the g-gate rows of
      W/b, and h is stored HALVED (hT = h/2, all h-consuming weights 2x).
    Fused scalar_tensor_tensor ops absorb the -0.5/*2 corrections:
      ig/2 = (sig(2g) - 0.5) * sig(i)        [one stt, PSUM+SBUF]
      c    = 2*(ig/2)^T + sig(f)^T * c       [mul + stt]
      h/2  = (sig(2c) - 0.5) * sig(o)^T      [one stt]
  - (sf, so, ig) sit contiguously in one [96, 512] SBUF tile, so FOUR
    paired PE transposes (ident96) move all three into h-major layout.
  - Encoder: x_t rides as a 5th column-tiled accumulation chunk
    ([xn_t|1|0...] stationary, [W_ih.T;b;0] moving). Decoder: projection
    folded into recurrent weights; bias DMA-prefilled into PSUM.
  - Decoder h/2 spooled to DRAM in bf16; batched GEMM + RevIN denorm.
"""

import os
import sys

_TRN_REPO = "/opt/trn_rl_repo"
if _TRN_REPO not in sys.path:
    sys.path.insert(0, _TRN_REPO)

import numpy as np

import concourse.bass as bass
import concourse.bacc as bacc
import concourse.tile as tile
from concourse import mybir
from concourse.bass_utils import run_bass_kernel_spmd
from concourse.masks import make_identity

NCORES = 8
B, D, H = 256, 16, 512
T_IN, T_OUT = 512, 200
BL = B // NCORES          # 32 batch per core
G = 4 * H                 # 2048 gate width
KC = H // 128             # 4 contraction chunks
EPS = 1e-5
F32 = mybir.dt.float32
BF = mybir.dt.bfloat16
AF = mybir.ActivationFunctionType
OP = mybir.AluOpType

# col-group order within the 2048-wide gate tile (torch gate order i,f,g,o)
# quadrant q holds gate WORD[q]: [f, o, i, g]
WORD = [1, 3, 0, 2]


def build_program(ti=T_IN, to=T_OUT, u_enc=8, u_dec=8, reps=1):
    """Build + compile the SPMD single-core program. Returns nc.

    reps > 1 wraps encoder+decoder in an outer hardware loop (for timing
    only; every rep recomputes identical state)."""
    assert ti % u_enc == 0 and to % u_dec == 0
    nc = bacc.Bacc("TRN2", target_bir_lowering=False, debug=False,
                   num_devices=NCORES)

    def din(name, shape, dt=F32):
        return nc.dram_tensor(name, shape, dt, kind="ExternalInput").ap()

    x_d = din("x", (BL, D, ti))
    wenc_d = din("wenc", (128, KC, G // 2))  # bf16 pairs packed in fp32 view
    wdec_d = din("wdec", (128, KC, G // 2))
    wxb_d = din("wxb", (128, G // 2))       # [W_ih.T ; b ; 0] (bf16 packed)
    wxbd_d = din("wxbd", (128, G // 2))     # [0 ; b_dec ; 0] (bf16 packed)
    wp_d = din("wp", (128, KC, D // 2))     # 2*W_proj.T chunks (bf16 packed)
    sc1_d = din("sc1", (D, 1))              # b_proj - rev_b
    sc2_d = din("sc2", (D, 1))              # 1/(rev_w + EPS^2)
    revw_d = din("revw", (D, 1))
    revb_d = din("revb", (D, 1))
    y_d = nc.dram_tensor("y", (BL, D, to), F32, kind="ExternalOutput").ap()

    from contextlib import ExitStack
    with tile.TileContext(nc) as tc, ExitStack() as ctx:
        const = ctx.enter_context(tc.tile_pool(name="const", bufs=1))
        state = ctx.enter_context(tc.tile_pool(name="state", bufs=1))
        sg = ctx.enter_context(tc.tile_pool(name="sg", bufs=2))
        xc = ctx.enter_context(tc.tile_pool(name="xc", bufs=3))
        pp = ctx.enter_context(tc.tile_pool(name="pp", bufs=1, space="PSUM"))
        dram = ctx.enter_context(tc.tile_pool(name="dram", bufs=1,
                                              space="DRAM"))
        xctx = ctx.enter_context(ExitStack())
        xpool = xctx.enter_context(tc.tile_pool(name="xp", bufs=1))

        # ---- constants / weights in SBUF ----
        wenc = const.tile([128, KC, G], BF)
        nc.sync.dma_start(wenc, wenc_d.bitcast(BF))
        wdec = const.tile([128, KC, G], BF)
        nc.sync.dma_start(wdec, wdec_d.bitcast(BF))
        wxb = const.tile([128, G], BF)
        nc.sync.dma_start(wxb, wxb_d.bitcast(BF))
        wxbd = const.tile([128, G], BF)
        nc.sync.dma_start(wxbd, wxbd_d.bitcast(BF))
        wp = const.tile([128, KC, D], BF)
        nc.sync.dma_start(wp, wp_d.bitcast(BF))
        sc1 = const.tile([D, 1], F32)
        nc.sync.dma_start(sc1, sc1_d)
        sc2 = const.tile([D, 1], F32)
        nc.sync.dma_start(sc2, sc2_d)
        revw = const.tile([D, 1], F32)
        nc.sync.dma_start(revw, revw_d)
        revb = const.tile([D, 1], F32)
        nc.sync.dma_start(revb, revb_d)
        ident96 = const.tile([96, 96], BF)
        make_identity(nc, ident96)

        # ---- input + RevIN ----
        X = xpool.tile([D, BL, ti], F32)          # xn, partition = d
        nc.sync.dma_start(X, x_d.transpose([1, 0, 2]))
        # stats in natural [b, d, t] layout: 16 op pairs at 32-partition
        # occupancy instead of 32 pairs at 16; result bounced through DRAM
        # to transpose into d-major (SBUF APs cannot transpose partitions)
        Xbt = xpool.tile([BL, D, ti], F32)
        nc.sync.dma_start(Xbt, x_d)

        bnst = xpool.tile([BL, D, 6], F32)
        mvbt = xpool.tile([BL, D, 2], F32)
        for d in range(D):
            nc.vector.bn_stats(bnst[:, d, :], Xbt[:, d, :])
            nc.vector.bn_aggr(mvbt[:, d, :], bnst[:, d, :])
        mvs = dram.tile([D, BL, 2], F32)
        nc.sync.dma_start(mvs.transpose([1, 0, 2]), mvbt)
        mv = const.tile([D, BL, 2], F32)          # [:, :, 0]=mean [:, :, 1]=var
        nc.sync.dma_start(mv, mvs)
        epst = const.tile([D, 1], F32)
        nc.vector.memset(epst, EPS)
        stdev = const.tile([D, BL], F32)
        nc.scalar.activation(stdev, mv[:, :, 1], AF.Sqrt, bias=epst)
        rstd = const.tile([D, BL], F32)
        nc.vector.reciprocal(rstd, stdev)

        mean_b = mv[:, :, 0:1].broadcast_to((D, BL, ti))
        rstd_b = rstd.unsqueeze(2).broadcast_to((D, BL, ti))
        nc.vector.tensor_tensor(X, X, mean_b, op=OP.subtract)
        nc.vector.tensor_tensor(X, X, rstd_b, op=OP.mult)
        Xb = xpool.tile([D, BL, ti], BF)          # bf16, mm operand
        nc.vector.tensor_scalar(Xb, X, revw, revb, op0=OP.mult, op1=OP.add)

        # ---- state (k-halves in separate tiles so half-0 of the tail
        # unblocks next-step matmuls k=0,1 early) ----
        hThs = [state.tile([128, KC // 2, BL], BF, tag=f"hTh{i}",
                           name=f"hTh{i}") for i in range(2)]
        cThs = [state.tile([128, KC // 2, BL], F32, tag=f"cTh{i}",
                           name=f"cTh{i}") for i in range(2)]

        gts = [pp.tile([128, 512], F32, tag=f"gt{j}", name=f"gt{j}")
               for j in range(2)]
        # single-buffered: ig consumes sig(2g) immediately, so the next
        # step's sigmoid write only WARs against a long-done read
        sgp = [pp.tile([32, 256], F32, tag=f"sgph{h}", name=f"sgph{h}")
               for h in range(2)]
        sfoTs = [pp.tile([128, KC // 2, 96], BF, tag=f"sfoT{i}",
                         name=f"sfoT{i}") for i in range(2)]

        hbuf = dram.tile([128, KC, to, BL], BF)   # decoder h/2 spool

        def cell(w4, gt, sgpsum, xw, wxbr):
            """One LSTM step: 4 col-tiled h chunks + one x/bias chunk."""
            # x/bias chunk first: no dependence on h(t-1), so it streams
            # while the recurrent tail of the previous step is still running
            for cg in range(4):   # weights pre-permuted to (f,o,i,2g)
                nc.tensor.matmul(
                    gt[32 * cg:32 * cg + 32, :],
                    xw, wxbr[:, bass.ts(cg, 512)],
                    start=True, stop=False,
                    tile_position=(0, 32 * cg))
            for k in range(KC):
                for cg in range(4):
                    nc.tensor.matmul(
                        gt[32 * cg:32 * cg + 32, :],
                        hThs[k // 2][:, k % 2, :],
                        w4[:, k, bass.ts(cg, 512)],
                        start=False, stop=(k == KC - 1),
                        tile_position=(0, 32 * cg))
            for half in range(2):
                cols = bass.ds(256 * half, 256)
                S = sg.tile([96, 256], BF, tag=f"S{half}", name=f"S{half}")
                nc.scalar.activation(S, gt[0:96, cols], AF.Sigmoid)
                nc.scalar.activation(sgpsum[half], gt[96:128, cols],
                                     AF.Sigmoid)
                # ig/2 = (sig(2g) - 0.5) * sig(i)
                nc.vector.scalar_tensor_tensor(
                    S[64:96, :], sgpsum[half], -0.5, S[64:96, :],
                    op0=OP.add, op1=OP.mult)
                sfoT = sfoTs[half]
                for kk in range(KC // 2):
                    nc.tensor.transpose(sfoT[:, kk, :],
                                        S[:, bass.ts(kk, 128)], ident96)
                fcT = sg.tile([128, KC // 2, BL], F32, tag=f"fcT{half}",
                              name=f"fcT{half}")
                nc.vector.tensor_mul(fcT, sfoT[:, :, 0:32], cThs[half])
                nc.vector.scalar_tensor_tensor(
                    cThs[half], sfoT[:, :, 64:96], 2.0, fcT,
                    op0=OP.mult, op1=OP.add)
                tcT = sg.tile([128, KC // 2, BL], F32, tag=f"tcT{half}",
                              name=f"tcT{half}")
                nc.scalar.activation(tcT, cThs[half], AF.Sigmoid, scale=2.0)
                # h/2 = (sig(2c) - 0.5) * sig(o)^T
                nc.vector.scalar_tensor_tensor(
                    hThs[half], tcT, -0.5, sfoT[:, :, 32:64],
                    op0=OP.add, op1=OP.mult)

        # [xn_t | 1 | 0...] stationary buffers; rows 17.. stay 0, row 16=1
        NXB = 5
        ones = const.tile([1, BL], BF)
        nc.vector.memset(ones.bitcast(F32), float(np.uint32(0x3F803F80).view(np.float32)))
        xcurs = [state.tile([128, BL], BF, tag=f"xcur{j}", name=f"xcur{j}")
                 for j in range(NXB)]
        for j in range(NXB):
            nc.vector.memset(xcurs[j].bitcast(F32), 0.0)
            nc.sync.dma_start(xcurs[j][16:17, :], ones)

        def reset_state():
            for i in range(2):
                nc.vector.memset(hThs[i].bitcast(F32), 0.0)
                nc.vector.memset(cThs[i], 0.0)

        bsel = xcurs[NXB - 1]   # decoder: never DMAed -> [0..,1@16,..0]

        def enc_body(t, j):
            xcur = xcurs[j % (NXB - 1)]
            nc.sync.dma_start(xcur[0:D, :],
                              Xb[:, :, bass.ds(t, 1)].squeeze(2))
            cell(wenc, gts[j % 2], sgp, xcur, wxb)

        def dec_body(t, j):
            for i in range(2):
                nc.sync.dma_start(
                    hbuf[:, 2 * i:2 * i + 2, bass.ds(t, 1), :].squeeze(2),
                    hThs[i])
            cell(wdec, gts[j % 2], sgp, bsel, wxbd)

        def run_net():
            reset_state()
            if u_enc >= ti:
                for t in range(ti):
                    enc_body(t, t)
            else:
                with tc.For_i(0, ti // u_enc) as it:
                    for j in range(u_enc):
                        enc_body(it * u_enc + j, j)
            if u_dec >= to:
                for t in range(to):
                    dec_body(t, t)
            else:
                with tc.For_i(0, to // u_dec) as it:
                    for j in range(u_dec):
                        dec_body(it * u_dec + j, j)

        if reps == 1:
            run_net()
        else:
            with tc.For_i(0, reps):
                run_net()

        # ---- projection of all decoder h's + RevIN denorm ----
        xctx.close()                         # free X (96KB/partition)
        p3 = ctx.enter_context(tc.tile_pool(name="p3", bufs=2))
        TN = 16                              # timesteps per GEMM chunk
        proj = p3.tile([D, BL, to], F32, tag="proj", bufs=1)
        pps = pp.tile([D, BL, TN], F32, tag="pps")
        nchunks = (to + TN - 1) // TN
        for ch in range(nchunks):
            t0 = ch * TN
            tn = min(TN, to - t0)
            rh = p3.tile([128, KC, TN, BL], BF, tag="rh")
            nc.sync.dma_start(rh[:, :, :tn, :], hbuf[:, :, t0:t0 + tn, :])
            for k in range(KC):
                nc.tensor.matmul(
                    pps[:, :, :tn],
                    wp[:, k, :],
                    rh[:, k, :tn, :].transpose([0, 2, 1]),
                    start=(k == 0), stop=(k == KC - 1))
            nc.vector.tensor_scalar(proj[:, :, t0:t0 + tn], pps[:, :, :tn],
                                    sc1, sc2, op0=OP.add, op1=OP.mult)
        std_b = stdev.unsqueeze(2).broadcast_to((D, BL, to))
        mean_b2 = mv[:, :, 0:1].broadcast_to((D, BL, to))
        nc.vector.tensor_tensor(proj, proj, std_b, op=OP.mult)
        nc.vector.tensor_tensor(proj, proj, mean_b2, op=OP.add)
        nc.sync.dma_start(y_d.transpose([1, 0, 2]), proj)

    nc.compile()
    return nc


def _bfpack(a):
    """fp32 array -> bf16 values packed pairwise into a fp32-shaped array
    of half the last-dim... we instead keep same shape: view as bf16 via
    rounding; returns float32 ndarray whose bitcast to bf16 equals a."""
    import ml_dtypes
    bf = a.astype(ml_dtypes.bfloat16)
    u16 = bf.view(np.uint16)
    # pack pairs of bf16 along last axis into uint32 -> float32 view
    assert u16.shape[-1] % 2 == 0
    lo = u16[..., 0::2].astype(np.uint32)
    hi = u16[..., 1::2].astype(np.uint32)
    packed = (hi << 16) | lo
    return packed.view(np.float32)


def prep_inputs(x, W_ih, W_hh, b_ih, b_hh, W_proj, b_proj, rev_w, rev_b,
                ti=T_IN, to=T_OUT):
    """Host-side weight packing -> per-core input maps."""
    f = np.float32
    x = np.ascontiguousarray(np.asarray(x, f))
    W_ih = np.asarray(W_ih, f); W_hh = np.asarray(W_hh, f)
    b_ih = np.asarray(b_ih, f); b_hh = np.asarray(b_hh, f)
    W_proj = np.asarray(W_proj, f); b_proj = np.asarray(b_proj, f)
    rev_w = np.asarray(rev_w, f); rev_b = np.asarray(rev_b, f)

    def perm2(w):
        """[4H, ...] rows: torch (i,f,g,o) -> (f,o,i,2g)."""
        i, fg, g, o = (w[0:H], w[H:2 * H], w[2 * H:3 * H], w[3 * H:4 * H])
        return np.concatenate([fg, o, i, 2.0 * g], axis=0)

    def wlayout(w):  # [G, H] -> [128, KC, G]
        return np.ascontiguousarray(w.T.reshape(KC, 128, G).transpose(1, 0, 2))

    b_enc = b_ih + b_hh
    # h is stored halved -> all h-contractions get 2x (g rows get 4x total)
    wenc = wlayout(2.0 * perm2(W_hh))
    wdec = wlayout(2.0 * perm2(W_hh + W_ih @ W_proj))
    # encoder x/bias chunk: [2*? no: x not halved] rows 0..15 = W_ih.T
    # (permuted cols, g cols 2x), row 16 = bias, rows 17.. = 0
    Wihp = perm2(W_ih)            # [G, D]
    bp = perm2(b_enc[:, None])[:, 0]
    wxb = np.zeros((128, G), f)
    wxb[0:D] = Wihp.T
    wxb[D] = bp
    b_dec = b_enc + W_ih @ b_proj
    bdp = perm2(b_dec[:, None])[:, 0]
    wxbd = np.zeros((128, G), f)
    wxbd[D] = bdp
    wpk = np.ascontiguousarray(
        (2.0 * W_proj).T.reshape(KC, 128, D).transpose(1, 0, 2))
    sc1 = np.ascontiguousarray((b_proj - rev_b)[:, None])
    sc2 = np.ascontiguousarray((1.0 / (rev_w + EPS * EPS))[:, None])
    shared = dict(wenc=_bfpack(wenc), wdec=_bfpack(wdec),
                  wxb=_bfpack(wxb), wxbd=_bfpack(wxbd),
                  wp=_bfpack(wpk), sc1=sc1, sc2=sc2,
                  revw=np.ascontiguousarray(rev_w[:, None]),
                  revb=np.ascontiguousarray(rev_b[:, None]))
    nb = x.shape[0]
    bl = nb // NCORES
    return [dict(x=np.ascontiguousarray(x[c * bl:(c + 1) * bl]), **shared)
            for c in range(NCORES)]


_cached = {}


def _get_program(key, **kw):
    if key not in _cached:
        _cached[key] = build_program(**kw)
    return _cached[key]


def kernel(**inputs) -> np.ndarray:
    trace = bool(int(os.environ.get("LSTM_TRACE", "0")))
    if trace:
        try:  # NTFF profiling hook is not present in all axon containers
            from antenv.axon_hooks import get_axon_ntff_profile_hook  # noqa: F401
        except ImportError:
            trace = False
    nc = _get_program("full")
    in_maps = prep_inputs(**inputs)
    res = run_bass_kernel_spmd(
        nc, in_maps, core_ids=list(range(NCORES)),
        trace=trace, trace_cores=[0] if trace else None)
    if trace:
        kernel.last_result = res
    y = np.concatenate([res.results[c]["y"] for c in range(NCORES)], axis=0)
    return y
